# revision 14
# baseline (speedup 1.0000x reference)
"""Distributed Trainium2 Bass kernel for a dense-transformer attention block.

Sharding (8 NeuronCores): core cid = 4*b + g
  - b = batch index (B=2), g = kv-head group (N_KV_HEADS=4)
  - Each core: LN1(x[b]) -> its 4 query heads + its 1 kv head (column
    parallel wq/wk/wv), RoPE, causal GQA attention with pre-ictal bias,
    AllGather of per-group attention outputs (groups [0..3], [4..7]),
    LN2, column-parallel wo -> output columns [512g:512g+512].
  - Host concatenates the 8 output shards.

Compute dtype: bf16 matmul operands, f32 PSUM accumulation, f32 softmax/LN.
"""

import math
from contextlib import ExitStack

import numpy as np
import ml_dtypes

import concourse.bass as bass
import concourse.bacc as bacc
import concourse.mybir as mybir
import concourse.tile as tile
from concourse.bass_utils import run_bass_kernel_spmd

# Problem constants (hardcoded per spec nn_Attention_36120674959366)
B = 2
S = 2048
DIM = 2048
N_HEADS = 16
N_KV_HEADS = 4
HEAD_DIM = 128
NH_LOC = N_HEADS // N_KV_HEADS  # 4 q-heads per core
DQ_LOC = NH_LOC * HEAD_DIM      # 512
PRE_ICTAL_WINDOW = 10
PRE_ICTAL_BIAS = 2.0
LN_EPS = 1e-5
NEG_INF = -1e9

SQD = math.sqrt(HEAD_DIM)           # 11.3137085
INV_SQD = 1.0 / SQD
BIAS_SCALED = PRE_ICTAL_BIAS * SQD  # 22.627417
NEG_SCALED = NEG_INF * SQD          # -1.13137085e10

NT = S // 128                        # 16 tiles of 128 rows
NC = DIM // 128                      # 16 dim chunks

F32 = mybir.dt.float32
BF16 = mybir.dt.bfloat16

_CACHED = {}


def build_nc():
    nc = bacc.Bacc("TRN2", target_bir_lowering=False, debug=False, num_devices=8)

    # ---- kernel I/O (per-core shards; same graph on all 8 cores) ----
    xs = nc.dram_tensor("xs", [S, DIM], F32, kind="ExternalInput")
    wqT = nc.dram_tensor("wqT", [DIM, DQ_LOC], F32, kind="ExternalInput")
    wkT = nc.dram_tensor("wkT", [DIM, HEAD_DIM], F32, kind="ExternalInput")
    wvT = nc.dram_tensor("wvT", [DIM, HEAD_DIM], F32, kind="ExternalInput")
    woT = nc.dram_tensor("woT", [DIM, DQ_LOC], F32, kind="ExternalInput")
    ln1w = nc.dram_tensor("ln1w", [DIM], F32, kind="ExternalInput")
    ln1b = nc.dram_tensor("ln1b", [DIM], F32, kind="ExternalInput")
    ln2w = nc.dram_tensor("ln2w", [DIM], F32, kind="ExternalInput")
    ln2b = nc.dram_tensor("ln2b", [DIM], F32, kind="ExternalInput")
    labels = nc.dram_tensor("labels", [S], F32, kind="ExternalInput")
    cosT = nc.dram_tensor("cosT", [HEAD_DIM, S], BF16, kind="ExternalInput")
    sinT = nc.dram_tensor("sinT", [HEAD_DIM, S], BF16, kind="ExternalInput")
    ident = nc.dram_tensor("ident", [128, 128], BF16, kind="ExternalInput")
    ident32 = nc.dram_tensor("ident32", [128, 128], F32, kind="ExternalInput")
    pswap = nc.dram_tensor("pswap", [128, 128], BF16, kind="ExternalInput")
    out = nc.dram_tensor("out", [S, DQ_LOC], F32, kind="ExternalOutput")

    AF = mybir.ActivationFunctionType
    OP = mybir.AluOpType

    with tile.TileContext(nc) as tc, ExitStack() as st:
        pc = st.enter_context(tc.tile_pool(name="const", bufs=1))
        dr = st.enter_context(tc.tile_pool(name="dr", bufs=1, space="DRAM"))

        # ================= constants into SBUF =================
        ident_sb = pc.tile([128, 128], BF16, tag="ident")
        nc.sync.dma_start(out=ident_sb[:, :], in_=ident[:, :])
        ident32_sb = pc.tile([128, 128], F32, tag="ident32")
        nc.sync.dma_start(out=ident32_sb[:, :], in_=ident32[:, :])
        pswap_sb = pc.tile([128, 128], BF16, tag="pswap")
        nc.sync.dma_start(out=pswap_sb[:, :], in_=pswap[:, :])
        cos_sb = pc.tile([128, S], BF16, tag="cos")
        nc.sync.dma_start(out=cos_sb[:, :], in_=cosT[:, :])
        sin_sb = pc.tile([128, S], BF16, tag="sin")
        nc.sync.dma_start(out=sin_sb[:, :], in_=sinT[:, :])

        def load_ln(t, name):
            tl = pc.tile([128, NC], F32, tag=name)
            nc.sync.dma_start(
                out=tl[:, :],
                in_=t.ap().rearrange("(c p) -> p c", p=128),
            )
            return tl

        w1_sb = load_ln(ln1w, "w1")
        b1_sb = load_ln(ln1b, "b1")
        w2_sb = load_ln(ln2w, "w2")
        b2_sb = load_ln(ln2b, "b2")

        # ============ seizure-label cumulative sums ============
        # csrow[0, 0] = 0 ; csrow[0, 1+j] = cumsum(labels)[j] ;
        # csrow[0, S+1 .. S+11] = cs[S-1] (clamp padding)
        lab_sb = pc.tile([1, S], F32, tag="lab")
        nc.sync.dma_start(out=lab_sb[:, :],
                          in_=labels.ap().rearrange("(o s) -> o s", o=1))
        zrow = pc.tile([1, S], F32, tag="zrow")
        nc.vector.memset(zrow[:, :], 0.0)
        csrow = pc.tile([1, S + 12], F32, tag="csrow")
        nc.vector.memset(csrow[:, 0:1], 0.0)
        nc.vector.tensor_tensor_scan(
            out=csrow[:, 1:S + 1],
            data0=lab_sb[:, :],
            data1=zrow[:, :],
            initial=0.0,
            op0=OP.add,
            op1=OP.add,
        )
        for j in range(11):
            nc.vector.tensor_copy(csrow[:, S + 1 + j:S + 2 + j], csrow[:, S:S + 1])

        # spill to DRAM, reload as [NT, 128] and transpose via PE:
        #   colv[p, kt] = csrow[128*kt + 11 + p] = cs[min(k+10, S-1)], k = 128*kt + p
        csbuf = dr.tile([S + 12], F32)
        nc.sync.dma_start(out=csbuf.rearrange("(o s) -> o s", o=1), in_=csrow[:, :])
        cs16 = pc.tile([NT, 128], F32, tag="cs16")
        nc.sync.dma_start(
            out=cs16[:, :],
            in_=csbuf[11:11 + S].rearrange("(t p) -> t p", p=128),
        )
        ones_col = pc.tile([1, 128], F32, tag="ones_col")
        nc.vector.memset(ones_col[:, :], 1.0)
        eps_sb = pc.tile([128, 1], F32, tag="eps")
        nc.vector.memset(eps_sb[:, :], LN_EPS)
        colv = pc.tile([128, NT], F32, tag="colv")

        # ============ M tiles (bias + causal, pre-scaled by sqrt(d)) ============
        n_mg = sum(len(range(max(0, 4 * a - 1), 4 * a + 4)) for a in range(4))
        mg = pc.tile([128, n_mg, 512], BF16, tag="mg")
        mg_idx = {}

        with tc.tile_pool(name="psA", bufs=1, space="PSUM") as psA:
            colv_ps = psA.tile([128, NT], F32, tag="colv_ps")
            nc.tensor.matmul(colv_ps[:, :], lhsT=cs16[:, :],
                             rhs=ident32_sb[0:NT, 0:NT], start=True, stop=True)
            nc.scalar.activation(colv[:, :], colv_ps[:, :], AF.Copy)

            nc.gpsimd.memset(mg[:, :, :], 0.0)

            def rowv_bcast(t):
                rb = psA.tile([128, 128], F32, tag="rb_ps", bufs=2)
                nc.tensor.matmul(rb[:, :], lhsT=ones_col[:, :],
                                 rhs=csrow[:, 128 * t:128 * t + 128],
                                 start=True, stop=True)
                return rb

            mslot = 0
            for a in range(4):
                for kt in range(max(0, 4 * a - 1), 4 * a + 4):
                    mg_idx[(a, kt)] = mslot
                    if 4 * a <= kt <= 4 * a + 3:  # diag: qtile t == kt
                        j = kt - 4 * a
                        rb = rowv_bcast(kt)
                        sl = mg[:, mslot, 128 * j:128 * j + 128]
                        nc.vector.tensor_scalar(
                            out=sl, in0=rb[:, :],
                            scalar1=colv[:, kt:kt + 1], scalar2=BIAS_SCALED,
                            op0=OP.is_lt, op1=OP.mult,
                        )
                        nc.gpsimd.affine_select(
                            out=sl, in_=sl,
                            compare_op=OP.is_ge, fill=NEG_SCALED,
                            base=0, channel_multiplier=-1, pattern=[[1, 128]],
                        )
                    tprev = kt + 1
                    if 4 * a <= tprev <= 4 * a + 3:  # prev: qtile t == kt + 1
                        j = tprev - 4 * a
                        rb = rowv_bcast(tprev)
                        sl = mg[:, mslot, 128 * j:128 * j + 128]
                        nc.vector.tensor_scalar(
                            out=sl, in0=rb[:, :],
                            scalar1=colv[:, kt:kt + 1], scalar2=BIAS_SCALED,
                            op0=OP.is_lt, op1=OP.mult,
                        )
                    mslot += 1

        with tc.tile_pool(name="qkv", bufs=1) as pqkv:
            qT = pqkv.tile([128, NH_LOC, S], BF16, tag="qT")
            kT = pqkv.tile([128, S], BF16, tag="kT")
            v_aug = pqkv.tile([128, NT, 132], BF16, tag="v_aug")

            with tc.tile_pool(name="ln1t", bufs=1) as p1:
                # ================= LN1 + transpose =================
                ln1T = p1.tile([128, NC, S], BF16, tag="ln1T")
                with tc.tile_pool(name="ln1tmp", bufs=1) as ptmp, \
                     tc.tile_pool(name="psB", bufs=1, space="PSUM") as psB:
                    for g4 in range(NT // 4):
                        xh_tiles = []
                        for j4 in range(4):
                            i = 4 * g4 + j4
                            xt = ptmp.tile([128, DIM], F32, tag="xt", bufs=3)
                            nc.sync.dma_start(out=xt[:, :],
                                              in_=xs[128 * i:128 * i + 128, :])
                            st6 = ptmp.tile([128, 4, 6], F32, tag="st6", bufs=2)
                            for a4 in range(4):
                                nc.vector.bn_stats(
                                    st6[:, a4, :],
                                    xt[:, 512 * a4:512 * a4 + 512])
                            mv = ptmp.tile([128, 2], F32, tag="mv", bufs=2)
                            nc.vector.bn_aggr(mv[:, :], st6[:, :, :])
                            std = ptmp.tile([128, 1], F32, tag="std", bufs=2)
                            nc.scalar.activation(std[:, :], mv[:, 1:2], AF.Sqrt,
                                                 bias=eps_sb[:, :])
                            rs = ptmp.tile([128, 1], F32, tag="rs", bufs=2)
                            nc.vector.reciprocal(rs[:, :], std[:, :])
                            nm = ptmp.tile([128, 1], F32, tag="nm", bufs=2)
                            nc.vector.scalar_tensor_tensor(
                                out=nm[:, :], in0=mv[:, 0:1], scalar=-1.0,
                                in1=rs[:, :], op0=OP.mult, op1=OP.mult)
                            xh = ptmp.tile([128, DIM], BF16, tag="xh", bufs=8)
                            nc.scalar.activation(xh[:, :], xt[:, :], AF.Identity,
                                                 scale=rs[:, :], bias=nm[:, :])
                            xh_tiles.append(xh)
                        for c in range(NC):
                            pt = psB.tile([128, 512], BF16, tag="pt", bufs=2)
                            for j4 in range(4):
                                nc.tensor.transpose(
                                    pt[:, 128 * j4:128 * j4 + 128],
                                    xh_tiles[j4][:, 128 * c:128 * c + 128],
                                    ident_sb[:, :])
                            nc.scalar.activation(
                                ln1T[:, c, 512 * g4:512 * g4 + 512],
                                pt[:, :], AF.Identity,
                                scale=w1_sb[:, c:c + 1], bias=b1_sb[:, c:c + 1])

                # ================= Q/K/V projections + RoPE =================
                with tc.tile_pool(name="qkvw", bufs=1) as pw, \
                     tc.tile_pool(name="rope", bufs=1) as pr, \
                     tc.tile_pool(name="psC", bufs=1, space="PSUM") as psC:
                    wq_sb = pw.tile([128, NC, DQ_LOC], BF16, tag="wq")
                    nc.gpsimd.dma_start(
                        out=wq_sb[:, :, :],
                        in_=wqT.ap().rearrange("(c p) n -> p c n", p=128))
                    wk_sb = pw.tile([128, NC, HEAD_DIM], BF16, tag="wk")
                    nc.gpsimd.dma_start(
                        out=wk_sb[:, :, :],
                        in_=wkT.ap().rearrange("(c p) n -> p c n", p=128))
                    wv_sb = pw.tile([128, NC, HEAD_DIM], BF16, tag="wv")
                    nc.gpsimd.dma_start(
                        out=wv_sb[:, :, :],
                        in_=wvT.ap().rearrange("(c p) n -> p c n", p=128))

                    def rope_block(dst, w_sb, h):
                        raw = pr.tile([128, S], BF16, tag="rope_raw", bufs=2)
                        swp = pr.tile([128, S], BF16, tag="rope_swp", bufs=2)
                        for sg in range(4):
                            pq = psC.tile([128, 512], F32, tag="pq", bufs=2)
                            for c in range(NC):
                                if h is None:
                                    lhsT = w_sb[:, c, :]
                                else:
                                    lhsT = w_sb[:, c, 128 * h:128 * h + 128]
                                nc.tensor.matmul(
                                    pq[:, :], lhsT=lhsT,
                                    rhs=ln1T[:, c, 512 * sg:512 * sg + 512],
                                    start=(c == 0), stop=(c == NC - 1))
                            nc.scalar.activation(raw[:, 512 * sg:512 * sg + 512],
                                                 pq[:, :], AF.Copy)
                        for sg in range(4):
                            pw2 = psC.tile([128, 512], F32, tag="pq", bufs=2)
                            nc.tensor.matmul(pw2[:, :], lhsT=pswap_sb[:, :],
                                             rhs=raw[:, 512 * sg:512 * sg + 512],
                                             start=True, stop=True)
                            nc.scalar.activation(swp[:, 512 * sg:512 * sg + 512],
                                                 pw2[:, :], AF.Copy)
                        t1 = pr.tile([128, S], BF16, tag="rope_t1", bufs=2)
                        nc.vector.tensor_mul(t1[:, :], raw[:, :], cos_sb[:, :])
                        t2 = pr.tile([128, S], BF16, tag="rope_t2", bufs=2)
                        nc.vector.tensor_mul(t2[:, :], swp[:, :], sin_sb[:, :])
                        nc.vector.tensor_add(dst, t1[:, :], t2[:, :])

                    for h in range(NH_LOC):
                        rope_block(qT[:, h, :], wq_sb, h)
                    rope_block(kT[:, :], wk_sb, None)

                    nc.gpsimd.memset(v_aug[:, :, 128:129], 1.0)
                    for i in range(NT):
                        pv = psC.tile([128, 128], F32, tag="pvproj", bufs=2)
                        for c in range(NC):
                            nc.tensor.matmul(
                                pv[:, :],
                                lhsT=ln1T[:, c, 128 * i:128 * i + 128],
                                rhs=wv_sb[:, c, :],
                                start=(c == 0), stop=(c == NC - 1))
                        nc.scalar.activation(v_aug[:, i, 0:128], pv[:, :], AF.Copy)
            # ln1T + proj weights released here

            # ================= attention =================
            bounce_in = dr.tile([S, DQ_LOC], BF16)
            with tc.tile_pool(name="attn", bufs=1) as pa, \
                 tc.tile_pool(name="psD", bufs=1, space="PSUM") as psD:
                attn_all = pa.tile([128, NT, DQ_LOC], BF16, tag="attn_all")
                for h in range(NH_LOC):
                    for a in range(4):
                        nkt = 4 * a + 4
                        pvp = [psD.tile([128, 132], F32, tag="pv_acc", bufs=4,
                                         name=f"pv_{h}_{a}_{jj}")
                               for jj in range(4)]
                        kt = 0
                        while kt < nkt:
                            mega = psD.tile([128, 1024], F32, tag="sc", bufs=2)
                            pair = [k2 for k2 in (kt, kt + 1) if k2 < nkt]
                            offs = []
                            for slot, k2 in enumerate(pair):
                                off = 128 * max(0, k2 - 4 * a)
                                offs.append(off)
                                reg = mega[:, 512 * slot + off:512 * slot + 512]
                                key = (a, k2)
                                if key in mg_idx:
                                    nc.tensor.matmul(
                                        reg, lhsT=ident_sb[:, :],
                                        rhs=mg[:, mg_idx[key], off:512],
                                        start=True, stop=False)
                                    sflag = False
                                else:
                                    sflag = True
                                nc.tensor.matmul(
                                    reg, lhsT=kT[:, 128 * k2:128 * k2 + 128],
                                    rhs=qT[:, h, 512 * a + off:512 * a + 512],
                                    start=sflag, stop=True)
                            pt = pa.tile([128, 1024], BF16, tag="pt_sm", bufs=3)
                            if len(pair) == 2 and offs[0] == 0 and offs[1] == 0:
                                nc.scalar.activation(pt[:, :], mega[:, :], AF.Exp,
                                                     scale=INV_SQD)
                            else:
                                for slot, k2 in enumerate(pair):
                                    off = offs[slot]
                                    nc.scalar.activation(
                                        pt[:, 512 * slot + off:512 * slot + 512],
                                        mega[:, 512 * slot + off:512 * slot + 512],
                                        AF.Exp, scale=INV_SQD)
                            for slot, k2 in enumerate(pair):
                                for j in range(max(0, k2 - 4 * a), 4):
                                    nc.tensor.matmul(
                                        pvp[j][:, 0:129],
                                        lhsT=pt[:, 512 * slot + 128 * j:
                                                512 * slot + 128 * j + 128],
                                        rhs=v_aug[:, k2, 0:129],
                                        start=(k2 == 0), stop=(k2 == 4 * a + j),
                                        skip_group_check=True)
                            kt += len(pair)
                        for j in range(4):
                            t = 4 * a + j
                            rcp = pa.tile([128, 1], F32, tag="rcp", bufs=4,
                                          name=f"rcp_{h}_{a}_{j}")
                            nc.vector.reciprocal(rcp[:, :], pvp[j][:, 128:129])
                            nc.vector.tensor_single_scalar(
                                out=attn_all[:, t, 128 * h:128 * h + 128],
                                in_=pvp[j][:, 0:128],
                                scalar=rcp[:, :],
                                op=OP.mult)

                nc.sync.dma_start(
                    out=bounce_in.rearrange("(t p) n -> p t n", p=128),
                    in_=attn_all[:, :, :])

        # ================= AllGather =================
        bounce_out = dr.tile([4, S, DQ_LOC], BF16)
        nc.gpsimd.collective_compute(
            "AllGather",
            mybir.AluOpType.bypass,
            replica_groups=[[0, 1, 2, 3], [4, 5, 6, 7]],
            ins=[bounce_in[:, :].opt()],
            outs=[bounce_out[:, :, :].opt()],
        )

        # ================= LN2 + wo =================
        with tc.tile_pool(name="ln2", bufs=1) as p2, \
             tc.tile_pool(name="psE", bufs=1, space="PSUM") as psE:
            wo_sb = p2.tile([128, NC, DQ_LOC], BF16, tag="wo")
            nc.gpsimd.dma_start(
                out=wo_sb[:, :, :],
                in_=woT.ap().rearrange("(c p) n -> p c n", p=128))

            af_tiles = []
            mv2 = p2.tile([128, NT, 2], F32, tag="mv2")
            for t in range(NT):
                af = p2.tile([128, DIM], BF16, tag="af", bufs=NT)
                for gp in range(4):
                    nc.sync.dma_start(
                        out=af[:, 512 * gp:512 * gp + 512],
                        in_=bounce_out[gp, 128 * t:128 * t + 128, :])
                st6b = p2.tile([128, 4, 6], F32, tag="st6b", bufs=2)
                for a4 in range(4):
                    nc.vector.bn_stats(st6b[:, a4, :],
                                       af[:, 512 * a4:512 * a4 + 512])
                nc.vector.bn_aggr(mv2[:, t, :], st6b[:, :, :])
                af_tiles.append(af)
            std2 = p2.tile([128, NT], F32, tag="std2")
            nc.scalar.activation(std2[:, :], mv2[:, :, 1], AF.Sqrt,
                                 bias=eps_sb[:, :])
            rs2 = p2.tile([128, NT], F32, tag="rs2")
            nc.vector.reciprocal(rs2[:, :], std2[:, :])
            nm2 = p2.tile([128, NT], F32, tag="nm2")
            nc.vector.scalar_tensor_tensor(
                out=nm2[:, :], in0=mv2[:, :, 0], scalar=-1.0, in1=rs2[:, :],
                op0=OP.mult, op1=OP.mult)

            for g4 in range(NT // 4):
                xh2_tiles = []
                for j4 in range(4):
                    t = 4 * g4 + j4
                    xh2 = p2.tile([128, DIM], BF16, tag="xh2", bufs=6)
                    nc.scalar.activation(xh2[:, :], af_tiles[t][:, :], AF.Identity,
                                         scale=rs2[:, t:t + 1],
                                         bias=nm2[:, t:t + 1])
                    xh2_tiles.append(xh2)
                ln2T = p2.tile([128, NC, 512], BF16, tag="ln2T", bufs=2)
                for c in range(NC):
                    pt2 = psE.tile([128, 512], BF16, tag="pt2", bufs=2)
                    for j4 in range(4):
                        nc.tensor.transpose(pt2[:, 128 * j4:128 * j4 + 128],
                                            xh2_tiles[j4][:, 128 * c:128 * c + 128],
                                            ident_sb[:, :])
                    nc.scalar.activation(ln2T[:, c, :], pt2[:, :], AF.Identity,
                                         scale=w2_sb[:, c:c + 1],
                                         bias=b2_sb[:, c:c + 1])
                for j4 in range(4):
                    t = 4 * g4 + j4
                    po = psE.tile([128, 512], F32, tag="po", bufs=2)
                    for c in range(NC):
                        nc.tensor.matmul(po[:, :],
                                         lhsT=ln2T[:, c, 128 * j4:128 * j4 + 128],
                                         rhs=wo_sb[:, c, :],
                                         start=(c == 0), stop=(c == NC - 1))
                    osb = p2.tile([128, DQ_LOC], F32, tag="osb", bufs=3)
                    nc.vector.tensor_copy(osb[:, :], po[:, :])
                    nc.sync.dma_start(out=out[128 * t:128 * t + 128, :],
                                      in_=osb[:, :])

    nc.compile()
    return nc


def _prep_inputs(x, freqs_cis, seizure_labels, wq, wk, wv, wo,
                 ln1_w, ln1_b, ln2_w, ln2_b):
    bf16 = ml_dtypes.bfloat16
    cos = np.asarray(freqs_cis[..., 0], dtype=np.float32)  # [S, 64]
    sin = np.asarray(freqs_cis[..., 1], dtype=np.float32)
    cosT = np.ascontiguousarray(np.repeat(cos.T, 2, axis=0), dtype=bf16)
    sgn = np.where(np.arange(HEAD_DIM) % 2 == 0, -1.0, 1.0).astype(np.float32)
    sinT = np.ascontiguousarray(np.repeat(sin.T, 2, axis=0) * sgn[:, None],
                                dtype=bf16)
    ident = np.eye(128, dtype=bf16)
    ident32 = np.eye(128, dtype=np.float32)
    psw = np.zeros((128, 128), dtype=np.float32)
    idx = np.arange(128)
    psw[idx ^ 1, idx] = 1.0  # out[m, s] = sum_k psw[k, m] * in[k, s] = in[m^1, s]
    psw = psw.astype(bf16)

    in_maps = []
    for cid in range(8):
        b, g = divmod(cid, 4)
        in_maps.append({
            "xs": np.ascontiguousarray(x[b], dtype=np.float32),
            "wqT": np.ascontiguousarray(wq[DQ_LOC * g:DQ_LOC * (g + 1), :].T,
                                        dtype=np.float32),
            "wkT": np.ascontiguousarray(wk[HEAD_DIM * g:HEAD_DIM * (g + 1), :].T,
                                        dtype=np.float32),
            "wvT": np.ascontiguousarray(wv[HEAD_DIM * g:HEAD_DIM * (g + 1), :].T,
                                        dtype=np.float32),
            "woT": np.ascontiguousarray(wo[DQ_LOC * g:DQ_LOC * (g + 1), :].T,
                                        dtype=np.float32),
            "ln1w": np.ascontiguousarray(ln1_w, dtype=np.float32),
            "ln1b": np.ascontiguousarray(ln1_b, dtype=np.float32),
            "ln2w": np.ascontiguousarray(ln2_w, dtype=np.float32),
            "ln2b": np.ascontiguousarray(ln2_b, dtype=np.float32),
            "labels": np.ascontiguousarray(seizure_labels[b], dtype=np.float32),
            "cosT": cosT, "sinT": sinT,
            "ident": ident, "ident32": ident32, "pswap": psw,
        })
    return in_maps


def run(inputs, trace=False, trace_cores=None):
    x = np.asarray(inputs["x"])
    mask = np.asarray(inputs["mask"])
    # this kernel specializes the additive mask to the causal prefill mask
    causal = np.where(np.tril(np.ones((S, S), dtype=bool)), 0.0, NEG_INF
                      ).astype(np.float32)
    if not np.array_equal(mask, causal):
        raise NotImplementedError("kernel specialized for causal prefill mask")

    in_maps = _prep_inputs(
        x, np.asarray(inputs["freqs_cis"]), np.asarray(inputs["seizure_labels"]),
        np.asarray(inputs["wq"]), np.asarray(inputs["wk"]),
        np.asarray(inputs["wv"]), np.asarray(inputs["wo"]),
        np.asarray(inputs["ln1_w"]), np.asarray(inputs["ln1_b"]),
        np.asarray(inputs["ln2_w"]), np.asarray(inputs["ln2_b"]))

    if "nc" not in _CACHED:
        _CACHED["nc"] = build_nc()
    nc = _CACHED["nc"]

    kw = {}
    if trace:
        kw = dict(trace=True,
                  trace_cores=trace_cores if trace_cores is not None else [0])
    res = run_bass_kernel_spmd(nc, in_maps, core_ids=list(range(8)), **kw)

    shards = [res.results[cid]["out"] for cid in range(8)]
    full = np.empty((B, S, DIM), dtype=np.float32)
    for cid in range(8):
        b, g = divmod(cid, 4)
        full[b, :, DQ_LOC * g:DQ_LOC * (g + 1)] = shards[cid]
    return full, res


def kernel(**inputs) -> np.ndarray:
    out, _ = run(inputs, trace=False)
    return out
